# revision 1
# baseline (speedup 1.0000x reference)
"""Trainium2 Bass kernel for the guided dynamic-conv CNN (nn_KernelLearningGuide).

Sharding: 8 cores = 4 images x 2 row-halves (64 rows each). Each core gets a
host-padded input slab (68 rows x 130 cols incl. conv padding + halo rows) so
every 3x3 conv is a bank of matmuls over flat pixel slices at tap offsets.
The only cross-core quantity (global mean-pool feeding conv22) is a [128]
pairwise AllReduce. All matmul operands are fp16 (full PE rate on TRN2,
~1e-3 rounding), accumulation in fp32 PSUM; BN scales folded into weights
host-side.
"""

import sys

if "/opt/trn_rl_repo" not in sys.path:
    sys.path.insert(0, "/opt/trn_rl_repo")

import numpy as np

# ---- problem constants (hardcoded per spec) ----
B, Ci, Cg, H, W = 4, 128, 64, 128, 128
N_CORES = 8
HALF = H // 2          # 64 rows per core
WP = W + 2             # 130 padded cols
SROWS = HALF + 4       # 68 slab rows (2 halo+pad rows each side)
AROWS = HALF + 2       # 66 stage-A rows (output rows + 1 halo row each side)
PAD = 2                # extra flat elements at each slab end for tap overhang
NF_IN = SROWS * WP + 2 * PAD   # 8844 input slab flat length
NF_A = AROWS * WP              # 8580 stage-A flat length
PADB = WP + 1                  # 131 bn2out flat pad each side (conv3 overhang)
NCHUNK = 512
EPS = 1e-5

_CACHE = {}


def _taps():
    # tap t = di*3+dj ; flat offset into input slab for stage-A position f:
    # in_flat = f + di*WP + dj - 1  (before +PAD base shift)
    return [(di, dj) for di in range(3) for dj in range(3)]


def _prep_params(w11, b11, w12, b12, w21, b21, w22, b22,
                 g1, be1, m1, v1, g2, be2, m2, v2, w3, g3, be3, m3, v3):
    """Host-side parameter re-layouts + BN folding. Returns dict of np arrays."""
    f16 = np.float16
    s1 = (g1 / np.sqrt(v1 + EPS)).astype(np.float64)
    bias1 = (be1 - m1 * g1 / np.sqrt(v1 + EPS)).astype(np.float32)
    s2 = (g2 / np.sqrt(v2 + EPS)).astype(np.float64)
    bias2 = (be2 - m2 * g2 / np.sqrt(v2 + EPS)).astype(np.float32)
    s3 = (g3 / np.sqrt(v3 + EPS)).astype(np.float64)
    bias3 = (be3 - m3 * g3 / np.sqrt(v3 + EPS)).astype(np.float32)

    def conv3x3_lhst(w):  # w [Mout, K, 3, 3] -> [K, 9, Mout]
        return np.ascontiguousarray(np.transpose(w, (1, 2, 3, 0)).reshape(
            w.shape[1], 9, w.shape[0]))

    # conv11 / conv21: input-channel part (K=128) and guidance part (K=64)
    def split_conv(w):
        wi = conv3x3_lhst(w[:, :Ci])                  # [128, 9, 128]
        wg = conv3x3_lhst(w[:, Ci:])                  # [64, 9, 128]
        # guidance pair-packed: pair p uses taps (3p, 3p+1); stacked K=128 where
        # rows 64:128 multiply G2a's bottom half (G shifted by (0,1)).
        wgp = np.zeros((128, 3, 128), np.float32)
        wgs = np.zeros((128, 3, 128), np.float32)
        for p in range(3):
            wgp[:64, p] = wg[:, 3 * p]
            wgp[64:, p] = wg[:, 3 * p + 1]
            wgs[:64, p] = wg[:, 3 * p + 2]           # tap (p,2); bottom rows 0
        return wi, wgp, wgs

    w11i, w11gp, w11gs = split_conv(w11)
    w21i, w21gp, w21gs = split_conv(w21)

    # conv12 (1x1): w12 [Ci*9, Ci] -> lhsT [K=128, 9, 128] with BN1 scale folded
    w12m = w12.reshape(Ci, 9, Ci).astype(np.float64)      # [c, t, k]
    w12m = w12m * s1[:, None, None]                       # fold s1 per out-ch c
    w12T = np.ascontiguousarray(np.transpose(w12m, (2, 1, 0)))  # [k, t, c]
    b12T = np.ascontiguousarray(
        (b12.reshape(Ci, 9).astype(np.float64) * s1[:, None]).astype(np.float32))

    # conv22: w22 [Ci*Ci, Ci]; fold /(H*W) mean and BN2 scale s2 (per out-ch i)
    w22m = w22.reshape(Ci, Ci, Ci).astype(np.float64)     # [i, j, k]
    w22m = w22m * (s2[:, None, None] / (H * W))
    w22pp = np.ascontiguousarray(
        np.transpose(w22m, (2, 1, 0)).reshape(Ci, Ci * Ci))  # [k, (j,i)]
    b22T = np.ascontiguousarray(
        (b22.reshape(Ci, Ci).astype(np.float64) * s2[:, None]).T.astype(np.float32))
    # b22T[j, i] = b22[i*Ci+j] * s2[i]

    # conv3: fold BN3 scale s3 per out-channel m
    w3m = w3.astype(np.float64) * s3[:, None, None, None]
    w3T = conv3x3_lhst(w3m)                               # [128, 9, 128]

    return dict(
        w11i=w11i.astype(f16), w11gp=w11gp.astype(f16), w11gs=w11gs.astype(f16),
        w21i=w21i.astype(f16), w21gp=w21gp.astype(f16), w21gs=w21gs.astype(f16),
        w12T=w12T.astype(f16), b12T=b12T,
        w22pp=w22pp.astype(f16), b22T=b22T,
        w3T=w3T.astype(f16),
        b11=np.ascontiguousarray(b11.astype(np.float32)[:, None]),
        b21=np.ascontiguousarray(b21.astype(np.float32)[:, None]),
        bias1=np.ascontiguousarray(bias1[:, None]),
        bias2=np.ascontiguousarray(bias2[:, None]),
        bias3=np.ascontiguousarray(bias3[:, None]),
    )


def _prep_slabs(input, weight):
    """Per-core input/guidance slabs, flattened + PAD elements at both ends."""
    xp = np.pad(input, ((0, 0), (0, 0), (2, 2), (1, 1)))    # [B, Ci, 132, 130]
    gp = np.pad(weight, ((0, 0), (0, 0), (2, 2), (1, 1)))   # [B, Cg, 132, 130]
    slabs = []
    for core in range(N_CORES):
        b, half = core // 2, core % 2
        r0 = half * HALF
        xs = xp[b, :, r0:r0 + SROWS].reshape(Ci, -1)
        gs = gp[b, :, r0:r0 + SROWS].reshape(Cg, -1)
        z2x = np.zeros((Ci, PAD), np.float32)
        z2g = np.zeros((Cg, PAD), np.float32)
        xflat = np.concatenate([z2x, xs, z2x], axis=1)       # [128, 8844]
        gflat = np.concatenate([z2g, gs, z2g], axis=1)       # [64, 8844]
        # G2a: top = G, bottom = G shifted by one flat element (tap Δ=(0,1))
        g2a = np.empty((128, NF_IN), np.float32)
        g2a[:64] = gflat
        g2a[64:, :-1] = gflat[:, 1:]
        g2a[64:, -1] = 0.0
        mask = np.zeros((128, 2), np.float32)
        mask[:, 0] = 0.0 if half == 0 else 1.0   # A-row 0 (image row r0-1)
        mask[:, 1] = 1.0 if half == 0 else 0.0   # A-row 65 (image row r0+64)
        slabs.append((np.ascontiguousarray(xflat),
                      np.ascontiguousarray(g2a),
                      mask))
    return slabs


def _build_nc():
    import concourse.bass as bass
    import concourse.mybir as mybir
    import concourse.tile as tile
    from concourse import bacc

    f16, f32 = mybir.dt.float16, mybir.dt.float32
    AF = mybir.ActivationFunctionType
    nc = bacc.Bacc("TRN2", target_bir_lowering=False, debug=False,
                   num_devices=N_CORES)

    # ---- DRAM I/O ----
    xd = nc.dram_tensor("x", [Ci, NF_IN], f32, kind="ExternalInput")
    gd = nc.dram_tensor("g2a", [128, NF_IN], f32, kind="ExternalInput")
    maskd = nc.dram_tensor("mask", [128, 2], f32, kind="ExternalInput")
    wd = {}
    for name, shape, dt in (
        ("w11i", [128, 9, 128], f16), ("w11gp", [128, 3, 128], f16),
        ("w11gs", [128, 3, 128], f16),
        ("w21i", [128, 9, 128], f16), ("w21gp", [128, 3, 128], f16),
        ("w21gs", [128, 3, 128], f16),
        ("w12T", [128, 9, 128], f16), ("b12T", [128, 9], f32),
        ("w22pp", [128, Ci * Ci], f16), ("b22T", [128, 128], f32),
        ("w3T", [128, 9, 128], f16),
        ("b11", [128, 1], f32), ("b21", [128, 1], f32),
        ("bias1", [128, 1], f32), ("bias2", [128, 1], f32),
        ("bias3", [128, 1], f32),
    ):
        wd[name] = nc.dram_tensor(name, shape, dt, kind="ExternalInput")
    outd = nc.dram_tensor("out", [Ci, HALF, W], f16, kind="ExternalOutput")

    taps = _taps()
    chunks = [(s0, min(NCHUNK, NF_A - s0)) for s0 in range(0, NF_A, NCHUNK)]

    with tile.TileContext(nc) as tc:
        with (
            tc.tile_pool(name="wpool", bufs=1) as wpool,
            tc.tile_pool(name="slab", bufs=1) as slab,
            tc.tile_pool(name="conv", bufs=3) as convp,
            tc.tile_pool(name="work", bufs=3) as work,
            tc.tile_pool(name="w12p", bufs=4) as w12p,
            tc.tile_pool(name="dram", bufs=1, space="DRAM") as dram,
        ):
            # ---- load weights ----
            wsb = {}
            for name, t in wd.items():
                wt = wpool.tile(list(t.shape), t.dtype, tag=name)
                nc.sync.dma_start(wt[:], t.ap())
                wsb[name] = wt

            # ---- input slabs: DMA fp32 bands -> fp16 slabs ----
            xs = slab.tile([128, NF_IN], f16, tag="xs")
            xs2 = slab.tile([128, NF_IN], f16, tag="xs2")  # xs shifted by 1
            gs = slab.tile([128, NF_IN], f16, tag="gs")
            for dst, src, sh in ((xs, xd, 0), (xs2, xd, 1), (gs, gd, 0)):
                for s0 in range(0, NF_IN, 1024):
                    ln = min(1024, NF_IN - s0 - sh)
                    if ln <= 0:
                        continue
                    t32 = convp.tile([128, 1024], f32, tag="cvt")
                    nc.sync.dma_start(t32[:, :ln], src.ap()[:, s0 + sh:s0 + sh + ln])
                    nc.vector.tensor_copy(dst[:, s0:s0 + ln], t32[:, :ln])

            # stage-A big fp16 slabs
            w21o = slab.tile([128, NF_A], f16, tag="w21o")
            localp = slab.tile([128, NF_A], f16, tag="localp")
            bn2o = slab.tile([128, PADB + NF_A + PADB], f16, tag="bn2o")
            outsl = slab.tile([128, NF_A], f16, tag="outsl")

            def conv_3x3_cat(psum_pool, tag, s0, ln, wi, wgp, wgs):
                """15 accumulating tap-matmuls of conv11/conv21 for one chunk."""
                p = psum_pool.tile([128, NCHUNK], f32, tag=tag)
                first = True
                for t, (di, dj) in enumerate(taps):
                    off = PAD + s0 + di * WP + dj - 1
                    nc.tensor.matmul(p[:, :ln], wi[:, t, :],
                                     xs[:, off:off + ln],
                                     start=first, stop=False)
                    first = False
                for pr in range(3):
                    di, dj = pr, 0
                    off = PAD + s0 + di * WP + dj - 1
                    nc.tensor.matmul(p[:, :ln], wgp[:, pr, :],
                                     gs[:, off:off + ln],
                                     start=False, stop=False)
                for q in range(3):
                    di, dj = q, 2
                    off = PAD + s0 + di * WP + dj - 1
                    nc.tensor.matmul(p[:, :ln], wgs[:, q, :],
                                     gs[:, off:off + ln],
                                     start=False, stop=(q == 2))
                return p

            # ================= phase 1: conv21 -> w21o slab =================
            with tc.tile_pool(name="ps21", bufs=2, space="PSUM") as ps21:
                for s0, ln in chunks:
                    p = conv_3x3_cat(ps21, "p21", s0, ln,
                                     wsb["w21i"], wsb["w21gp"], wsb["w21gs"])
                    nc.scalar.activation(w21o[:, s0:s0 + ln], p[:, :ln],
                                         AF.Relu, bias=wsb["b21"][:, 0:1])

            # ---- pool partial (owned 64 rows x 128 cols) + AllReduce ----
            pool_part = work.tile([128, 1], f32, tag="poolp")
            own = w21o[:].rearrange("p (r c) -> p r c", c=WP)[:, 1:1 + HALF, 1:1 + W]
            nc.vector.reduce_sum(pool_part[:, 0:1], own,
                                 axis=mybir.AxisListType.XY)
            cin = dram.tile([128, 1], f32)
            cout = dram.tile([128, 1], f32)
            nc.gpsimd.dma_start(cin[:], pool_part[:])
            nc.gpsimd.collective_compute(
                "AllReduce", mybir.AluOpType.add,
                replica_groups=[[0, 1], [2, 3], [4, 5], [6, 7]],
                ins=[cin.opt()], outs=[cout.opt()],
            )
            pool_sum = work.tile([128, 1], f16, tag="pools")
            pool_f32 = work.tile([128, 1], f32, tag="poolf")
            nc.gpsimd.dma_start(pool_f32[:], cout[:])
            nc.vector.tensor_copy(pool_sum[:], pool_f32[:])

            # ============ phase 2: conv11 -> conv12 -> guided -> BN1 ========
            with (
                tc.tile_pool(name="ps11", bufs=2, space="PSUM") as ps11,
                tc.tile_pool(name="ps12", bufs=4, space="PSUM") as ps12,
            ):
                for s0, ln in chunks:
                    p11 = conv_3x3_cat(ps11, "p11", s0, ln,
                                       wsb["w11i"], wsb["w11gp"], wsb["w11gs"])
                    w11oc = convp.tile([128, NCHUNK], f16, tag="w11oc")
                    nc.scalar.activation(w11oc[:, :ln], p11[:, :ln],
                                         AF.Relu, bias=wsb["b11"][:, 0:1])
                    acc = convp.tile([128, NCHUNK], f16, tag="acc")
                    for t, (di, dj) in enumerate(taps):
                        p12 = ps12.tile([128, NCHUNK], f32, tag="p12")
                        nc.tensor.matmul(p12[:, :ln], wsb["w12T"][:, t, :],
                                         w11oc[:, :ln], start=True, stop=True)
                        w12oc = w12p.tile([128, NCHUNK], f16, tag="w12oc")
                        nc.scalar.activation(w12oc[:, :ln], p12[:, :ln],
                                             AF.Identity,
                                             bias=wsb["b12T"][:, t:t + 1])
                        xoff = PAD + s0 + di * WP + dj - 1
                        xop = xs[:, xoff:xoff + ln] if xoff % 2 == 0 \
                            else xs2[:, xoff - 1:xoff - 1 + ln]
                        if t == 0:
                            nc.vector.tensor_mul(acc[:, :ln], w12oc[:, :ln], xop)
                        else:
                            prod = convp.tile([128, NCHUNK], f16, tag="prod")
                            nc.vector.tensor_mul(prod[:, :ln], w12oc[:, :ln], xop)
                            nc.vector.tensor_add(acc[:, :ln], acc[:, :ln],
                                                 prod[:, :ln])
                    nc.scalar.activation(localp[:, s0:s0 + ln], acc[:, :ln],
                                         AF.Relu, bias=wsb["bias1"][:, 0:1])

            # ============ phase 3: conv22 (tiny) -> w22oT ====================
            w22tmp = dram.tile([128, 128], f32)
            with tc.tile_pool(name="ps22", bufs=2, space="PSUM") as ps22:
                for ci in range(32):
                    wc = work.tile([128, NCHUNK], f16, tag="w22c")
                    nc.sync.dma_start(
                        wc[:], wd["w22pp"].ap()[:, 512 * ci:512 * (ci + 1)])
                    p22 = ps22.tile([1, NCHUNK], f32, tag="p22")
                    nc.tensor.matmul(p22[:], pool_sum[:, 0:1], wc[:],
                                     start=True, stop=True)
                    p22s = work.tile([1, NCHUNK], f32, tag="p22s")
                    nc.scalar.copy(p22s[:], p22[:])
                    nc.sync.dma_start(
                        w22tmp[4 * ci:4 * ci + 4, :],
                        p22s[0:1, :].rearrange("o (a b) -> o a b", a=4))
            w22of = work.tile([128, 128], f32, tag="w22of")
            nc.sync.dma_start(w22of[:], w22tmp[:])
            nc.vector.tensor_add(w22of[:], w22of[:], wsb["b22T"][:, :])
            w22oT = work.tile([128, 128], f16, tag="w22oT")
            nc.vector.tensor_copy(w22oT[:], w22of[:])

            # ============ phase 4: bmm -> BN2 -> bn2o slab ==================
            with tc.tile_pool(name="psb", bufs=2, space="PSUM") as psb:
                for s0, ln in chunks:
                    pb = psb.tile([128, NCHUNK], f32, tag="pb")
                    nc.tensor.matmul(pb[:, :ln], w22oT[:, :],
                                     localp[:, s0:s0 + ln], start=True, stop=True)
                    nc.scalar.activation(bn2o[:, PADB + s0:PADB + s0 + ln],
                                         pb[:, :ln], AF.Relu,
                                         bias=wsb["bias2"][:, 0:1])

            # ---- boundary fixes on bn2o ----
            nc.vector.memset(bn2o[:, 0:PADB], 0.0)
            nc.vector.memset(bn2o[:, PADB + NF_A:], 0.0)
            rows = bn2o[:, PADB:PADB + NF_A].rearrange("p (r c) -> p r c", c=WP)
            nc.vector.memset(rows[:, :, 0:1], 0.0)
            nc.vector.memset(rows[:, :, WP - 1:WP], 0.0)
            msk = slab.tile([128, 2], f32, tag="msk")
            nc.sync.dma_start(msk[:], maskd.ap())
            for (ridx, mi) in ((0, 0), (AROWS - 1, 1)):
                rv = bn2o[:, PADB + ridx * WP:PADB + (ridx + 1) * WP]
                nc.vector.tensor_mul(
                    rv, rv, msk[:, mi:mi + 1].to_broadcast((128, WP)))

            # ============ phase 5: conv3 -> BN3 -> out slab =================
            with tc.tile_pool(name="ps3", bufs=2, space="PSUM") as ps3:
                for s0, ln in chunks:
                    p3 = ps3.tile([128, NCHUNK], f32, tag="p3")
                    first = True
                    for t, (di, dj) in enumerate(taps):
                        off = PADB + s0 + (di - 1) * WP + (dj - 1)
                        nc.tensor.matmul(p3[:, :ln], wsb["w3T"][:, t, :],
                                         bn2o[:, off:off + ln],
                                         start=first, stop=(t == 8))
                        first = False
                    nc.scalar.activation(outsl[:, s0:s0 + ln], p3[:, :ln],
                                         AF.Relu, bias=wsb["bias3"][:, 0:1])

            # ---- strided store of owned rows/cols ----
            ov = outsl[:].rearrange("p (r c) -> p r c", c=WP)[:, 1:1 + HALF, 1:1 + W]
            nc.sync.dma_start(outd.ap(), ov)

    nc.compile()
    return nc


def _get_nc():
    if "nc" not in _CACHE:
        _CACHE["nc"] = _build_nc()
    return _CACHE["nc"]


def kernel(**inputs) -> np.ndarray:
    from concourse.bass_utils import run_bass_kernel_spmd

    nc = _get_nc()
    params = _prep_params(**{k: np.asarray(v) for k, v in inputs.items()
                             if k not in ("input", "weight")})
    slabs = _prep_slabs(np.asarray(inputs["input"], np.float32),
                        np.asarray(inputs["weight"], np.float32))
    in_maps = []
    for core in range(N_CORES):
        xflat, g2a, mask = slabs[core]
        m = {"x": xflat, "g2a": g2a, "mask": mask}
        m.update(params)
        in_maps.append(m)
    res = run_bass_kernel_spmd(nc, in_maps, core_ids=list(range(N_CORES)),
                               **_CACHE.get("run_kwargs", {}))
    _CACHE["last_results"] = res
    out = np.empty((B, Ci, H, W), np.float32)
    for core in range(N_CORES):
        b, half = core // 2, core % 2
        out[b, :, half * HALF:(half + 1) * HALF, :] = res.results[core]["out"].astype(np.float32)
    return out



# revision 5
# speedup vs baseline: 1.0768x; 1.0768x over previous
"""Trainium2 Bass kernel for the guided dynamic-conv CNN (nn_KernelLearningGuide).

Sharding: 8 cores = 4 images x 2 row-halves (64 rows each). Each core gets
host-padded fp16 slabs (68 rows x 130 cols incl. conv padding + halo rows) so
every 3x3 conv is a bank of matmuls over flat pixel slices at tap offsets.
Guidance taps are pair-packed into stacked 128-row slabs (G|G<<1 and G|G<<WP)
so each 3x3-over-192-channels conv is 14 matmuls (the K=64 guidance taps ride
two-per-pass). The per-pixel guided conv spreads its elementwise work across
scalar (PSUM downcast+bias), vector and gpsimd. conv22 (pooled matvec) is
split across the core pair and recombined with an AllGather; the only other
cross-core exchange is the [128] mean-pool AllReduce. All matmul operands are
fp16 (full PE rate), fp32 PSUM accumulation; BN scales folded host-side.
"""

import sys

if "/opt/trn_rl_repo" not in sys.path:
    sys.path.insert(0, "/opt/trn_rl_repo")

import numpy as np

# ---- problem constants (hardcoded per spec) ----
B, Ci, Cg, H, W = 4, 128, 64, 128, 128
N_CORES = 8
HALF = H // 2          # 64 rows per core
WP = W + 2             # 130 padded cols
SROWS = HALF + 4       # 68 slab rows (2 halo+pad rows each side)
AROWS = HALF + 2       # 66 stage-A rows (output rows + 1 halo row each side)
PAD = 2                # extra flat elements at each slab end for tap overhang
NF_IN = SROWS * WP + 2 * PAD   # 8844 input slab flat length
NF_A = AROWS * WP              # 8580 stage-A flat length
PADB = WP + 1                  # 131 bn2out flat pad each side (conv3 overhang)
NCHUNK = 512
EPS = 1e-5

# phase-1 (conv21) only needs the 64 owned rows: flat [130, 8450)
P1_BASE = WP
P1_LEN = HALF * WP             # 8320

_CACHE = {}


def _taps():
    return [(di, dj) for di in range(3) for dj in range(3)]


def _prep_params(w11, b11, w12, b12, w21, b21, w22, b22,
                 g1, be1, m1, v1, g2, be2, m2, v2, w3, g3, be3, m3, v3):
    """Host-side parameter re-layouts + BN folding. Returns dict of np arrays
    (w22half is a 2-tuple: even-core half / odd-core half)."""
    f16 = np.float16
    s1 = (g1 / np.sqrt(v1 + EPS)).astype(np.float64)
    bias1 = (be1 - m1 * g1 / np.sqrt(v1 + EPS)).astype(np.float32)
    s2 = (g2 / np.sqrt(v2 + EPS)).astype(np.float64)
    bias2 = (be2 - m2 * g2 / np.sqrt(v2 + EPS)).astype(np.float32)
    s3 = (g3 / np.sqrt(v3 + EPS)).astype(np.float64)
    bias3 = (be3 - m3 * g3 / np.sqrt(v3 + EPS)).astype(np.float32)

    def conv3x3_lhst(w):  # w [Mout, K, 3, 3] -> [K, 9, Mout]
        return np.ascontiguousarray(np.transpose(w, (1, 2, 3, 0)).reshape(
            w.shape[1], 9, w.shape[0]))

    # conv11 / conv21: input part (K=128, 9 taps) and guidance part (K=64,
    # pair-packed into stacked-128 operands):
    #   wgp[p]: taps (p,0) top / (p,1) bottom   (rhs = gpa at d(p,0))
    #   wgw:    taps (0,2) top / (1,2) bottom   (rhs = gww at d(0,2))
    #   wgs:    tap (2,2) top / zero bottom     (rhs = gpa at d(2,2))
    def split_conv(w):
        wi = conv3x3_lhst(w[:, :Ci])                  # [128, 9, 128]
        wg = conv3x3_lhst(w[:, Ci:])                  # [64, 9, 128]
        wgp = np.zeros((128, 3, 128), np.float32)
        for p in range(3):
            wgp[:64, p] = wg[:, 3 * p]
            wgp[64:, p] = wg[:, 3 * p + 1]
        wgw = np.zeros((128, 1, 128), np.float32)
        wgw[:64, 0] = wg[:, 2]
        wgw[64:, 0] = wg[:, 5]
        wgs = np.zeros((128, 1, 128), np.float32)
        wgs[:64, 0] = wg[:, 8]
        return wi, wgp, wgw, wgs

    w11i, w11gp, w11gw, w11gs = split_conv(w11)
    w21i, w21gp, w21gw, w21gs = split_conv(w21)

    # conv12 (1x1): w12 [Ci*9, Ci] -> lhsT [K=128, 9, 128] with BN1 scale folded
    w12m = w12.reshape(Ci, 9, Ci).astype(np.float64)      # [c, t, k]
    w12m = w12m * s1[:, None, None]
    w12T = np.ascontiguousarray(np.transpose(w12m, (2, 1, 0)))  # [k, t, c]
    b12T = np.ascontiguousarray(
        (b12.reshape(Ci, 9).astype(np.float64) * s1[:, None]).astype(np.float32))

    # conv22: w22 [Ci*Ci, Ci]; fold /(H*W) mean and BN2 scale s2 (per out-ch i)
    w22m = w22.reshape(Ci, Ci, Ci).astype(np.float64)     # [i, j, k]
    w22m = w22m * (s2[:, None, None] / (H * W))
    w22pp = np.transpose(w22m, (2, 1, 0)).reshape(Ci, Ci * Ci)  # [k, (j,i)]
    w22halves = (np.ascontiguousarray(w22pp[:, :Ci * Ci // 2]).astype(f16),
                 np.ascontiguousarray(w22pp[:, Ci * Ci // 2:]).astype(f16))
    b22T = np.ascontiguousarray(
        (b22.reshape(Ci, Ci).astype(np.float64) * s2[:, None]).T.astype(np.float32))
    # b22T[j, i] = b22[i*Ci+j] * s2[i]

    # conv3: fold BN3 scale s3 per out-channel m
    w3m = w3.astype(np.float64) * s3[:, None, None, None]
    w3T = conv3x3_lhst(w3m)                               # [128, 9, 128]

    return dict(
        w11i=w11i.astype(f16), w11gp=w11gp.astype(f16),
        w11gw=w11gw.astype(f16), w11gs=w11gs.astype(f16),
        w21i=w21i.astype(f16), w21gp=w21gp.astype(f16),
        w21gw=w21gw.astype(f16), w21gs=w21gs.astype(f16),
        w12T=w12T.astype(f16), b12T=b12T,
        b22T=b22T,
        w3T=w3T.astype(f16),
        b11=np.ascontiguousarray(b11.astype(np.float32)[:, None]),
        b21=np.ascontiguousarray(b21.astype(np.float32)[:, None]),
        bias1=np.ascontiguousarray(bias1[:, None]),
        bias2=np.ascontiguousarray(bias2[:, None]),
        bias3=np.ascontiguousarray(bias3[:, None]),
    ), w22halves


def _prep_slabs(input, weight):
    """Per-core fp16 slabs: xs, xs2 (=xs<<1), gpa (G|G<<1), gww (G|G<<WP)."""
    f16 = np.float16
    xp = np.pad(input, ((0, 0), (0, 0), (2, 2), (1, 1)))    # [B, Ci, 132, 130]
    gp = np.pad(weight, ((0, 0), (0, 0), (2, 2), (1, 1)))   # [B, Cg, 132, 130]
    slabs = []
    for core in range(N_CORES):
        b, half = core // 2, core % 2
        r0 = half * HALF
        xsrow = xp[b, :, r0:r0 + SROWS].reshape(Ci, -1).astype(f16)
        gsrow = gp[b, :, r0:r0 + SROWS].reshape(Cg, -1).astype(f16)
        xflat = np.zeros((Ci, NF_IN), f16)
        xflat[:, PAD:PAD + SROWS * WP] = xsrow
        x2 = np.zeros((Ci, NF_IN), f16)
        x2[:, :-1] = xflat[:, 1:]
        gflat = np.zeros((Cg, NF_IN), f16)
        gflat[:, PAD:PAD + SROWS * WP] = gsrow
        gpa = np.zeros((128, NF_IN), f16)
        gpa[:64] = gflat
        gpa[64:, :-1] = gflat[:, 1:]
        gww = np.zeros((128, NF_IN), f16)
        gww[:64] = gflat
        gww[64:, :-WP] = gflat[:, WP:]
        mask = np.zeros((128, 2), np.float32)
        mask[:, 0] = 0.0 if half == 0 else 1.0   # A-row 0 (image row r0-1)
        mask[:, 1] = 1.0 if half == 0 else 0.0   # A-row 65 (image row r0+64)
        slabs.append((np.ascontiguousarray(xflat), np.ascontiguousarray(x2),
                      np.ascontiguousarray(gpa), np.ascontiguousarray(gww),
                      mask))
    return slabs


def _build_nc():
    import concourse.bass as bass
    import concourse.mybir as mybir
    import concourse.tile as tile
    from concourse import bacc

    f16, f32 = mybir.dt.float16, mybir.dt.float32
    AF = mybir.ActivationFunctionType
    nc = bacc.Bacc("TRN2", target_bir_lowering=False, debug=False,
                   num_devices=N_CORES)

    # ---- DRAM I/O ----
    xd = nc.dram_tensor("x", [Ci, NF_IN], f16, kind="ExternalInput")
    x2d = nc.dram_tensor("x2", [Ci, NF_IN], f16, kind="ExternalInput")
    gpad = nc.dram_tensor("gpa", [128, NF_IN], f16, kind="ExternalInput")
    gwwd = nc.dram_tensor("gww", [128, NF_IN], f16, kind="ExternalInput")
    maskd = nc.dram_tensor("mask", [128, 2], f32, kind="ExternalInput")
    wd = {}
    for name, shape, dt in (
        ("w11i", [128, 9, 128], f16), ("w11gp", [128, 3, 128], f16),
        ("w11gw", [128, 1, 128], f16), ("w11gs", [128, 1, 128], f16),
        ("w21i", [128, 9, 128], f16), ("w21gp", [128, 3, 128], f16),
        ("w21gw", [128, 1, 128], f16), ("w21gs", [128, 1, 128], f16),
        ("w12T", [128, 9, 128], f16), ("b12T", [128, 9], f32),
        ("b22T", [128, 128], f32),
        ("w3T", [128, 9, 128], f16),
        ("b11", [128, 1], f32), ("b21", [128, 1], f32),
        ("bias1", [128, 1], f32), ("bias2", [128, 1], f32),
        ("bias3", [128, 1], f32),
    ):
        wd[name] = nc.dram_tensor(name, shape, dt, kind="ExternalInput")
    w22hd = nc.dram_tensor("w22half", [128, Ci * Ci // 2], f16,
                           kind="ExternalInput")
    outd = nc.dram_tensor("out", [Ci, HALF, W], f16, kind="ExternalOutput")

    taps = _taps()
    chunks = [(s0, min(NCHUNK, NF_A - s0)) for s0 in range(0, NF_A, NCHUNK)]
    p1chunks = [(P1_BASE + s, min(NCHUNK, P1_LEN - s))
                for s in range(0, P1_LEN, NCHUNK)]

    with tile.TileContext(nc) as tc:
        with (
            tc.tile_pool(name="wpool", bufs=1) as wpool,
            tc.tile_pool(name="slab", bufs=1) as slab,
            tc.tile_pool(name="convp", bufs=3) as convp,
            tc.tile_pool(name="w12p", bufs=4) as w12p,
            tc.tile_pool(name="prodp", bufs=3) as prodp,
            tc.tile_pool(name="work", bufs=2) as work,
            tc.tile_pool(name="wcp", bufs=4) as wcp,
            tc.tile_pool(name="dram", bufs=1, space="DRAM") as dram,
        ):
            # ---- load weights (small, up-front) ----
            wsb = {}
            for name, t in wd.items():
                wt = wpool.tile(list(t.shape), t.dtype, tag=name)
                nc.sync.dma_start(wt[:], t.ap())
                wsb[name] = wt
            msk = wpool.tile([128, 2], f32, tag="msk")
            nc.sync.dma_start(msk[:], maskd.ap())

            # ---- input slabs: fp16 piece DMAs in consumption order ----
            xs = slab.tile([128, NF_IN], f16, tag="xs")
            xs2 = slab.tile([128, NF_IN], f16, tag="xs2")
            gpa = slab.tile([128, NF_IN], f16, tag="gpa")
            gww = slab.tile([128, NF_IN], f16, tag="gww")
            pieces = [0, 1280, 3840, 6400, NF_IN]
            for i in range(len(pieces) - 1):
                a, b = pieces[i], pieces[i + 1]
                for dst, src in ((xs, xd), (gpa, gpad), (gww, gwwd)):
                    nc.sync.dma_start(dst[:, a:b], src.ap()[:, a:b])
            for i in range(len(pieces) - 1):
                a, b = pieces[i], pieces[i + 1]
                nc.sync.dma_start(xs2[:, a:b], x2d.ap()[:, a:b])

            # stage-A big fp16 slabs
            w21o = slab.tile([128, P1_BASE + P1_LEN], f16, tag="w21o")
            localp = slab.tile([128, NF_A], f16, tag="localp")
            bn2o = slab.tile([128, PADB + NF_A + PADB], f16, tag="bn2o")
            outsl = slab.tile([128, NF_A], f16, tag="outsl")

            def conv14(psum_pool, tag, s0, ln, wi, wgp, wgw, wgs):
                """3x3 conv over cat(x,g): 9 input taps + 5 packed guidance."""
                p = psum_pool.tile([128, NCHUNK], f32, tag=tag)
                for t, (di, dj) in enumerate(taps):
                    off = PAD + s0 + di * WP + dj - 1
                    nc.tensor.matmul(p[:, :ln], wi[:, t, :],
                                     xs[:, off:off + ln],
                                     start=(t == 0), stop=False)
                for pr in range(3):
                    off = PAD + s0 + pr * WP - 1
                    nc.tensor.matmul(p[:, :ln], wgp[:, pr, :],
                                     gpa[:, off:off + ln],
                                     start=False, stop=False)
                off = PAD + s0 + 1
                nc.tensor.matmul(p[:, :ln], wgw[:, 0, :],
                                 gww[:, off:off + ln], start=False, stop=False)
                off = PAD + s0 + 2 * WP + 1
                nc.tensor.matmul(p[:, :ln], wgs[:, 0, :],
                                 gpa[:, off:off + ln], start=False, stop=True)
                return p

            # ================= phase 1: conv21 -> w21o (owned rows) =========
            with tc.tile_pool(name="ps21", bufs=2, space="PSUM") as ps21:
                for s0, ln in p1chunks:
                    p = conv14(ps21, "p21", s0, ln, wsb["w21i"], wsb["w21gp"],
                               wsb["w21gw"], wsb["w21gs"])
                    nc.scalar.activation(w21o[:, s0:s0 + ln], p[:, :ln],
                                         AF.Relu, bias=wsb["b21"][:, 0:1])

            # ---- pool (owned 64 rows x 128 cols) + pairwise AllReduce ----
            pool_part = work.tile([128, 1], f32, tag="poolp")
            own = w21o[:].rearrange("p (r c) -> p r c", c=WP)[:, 1:1 + HALF,
                                                             1:1 + W]
            nc.vector.reduce_sum(pool_part[:, 0:1], own,
                                 axis=mybir.AxisListType.XY)
            cin = dram.tile([128, 1], f32)
            cout = dram.tile([128, 1], f32)
            nc.gpsimd.dma_start(cin[:], pool_part[:])
            nc.gpsimd.collective_compute(
                "AllReduce", mybir.AluOpType.add,
                replica_groups=[[0, 1], [2, 3], [4, 5], [6, 7]],
                ins=[cin.opt()], outs=[cout.opt()],
            )
            pool_sum = work.tile([128, 1], f16, tag="pools")
            pool_f32 = work.tile([128, 1], f32, tag="poolf")
            nc.gpsimd.dma_start(pool_f32[:], cout[:])
            nc.vector.tensor_copy(pool_sum[:], pool_f32[:])

            # conv22 half result staging (this core computes 64 of 128 j-rows)
            w22stage = dram.tile([64, 128], f32)
            w22full = dram.tile([128, 128], f32)

            # ============ phase 2: conv11 -> conv12 -> guided -> BN1 ========
            # per tap: D = PSUM downcast+bias (scalar for taps 0-6, vector for
            # 7-8), M = product with shifted x (vector; taps 5-6 on gpsimd),
            # two accumulate chains (vector / gpsimd) joined at the end.
            with (
                tc.tile_pool(name="ps11", bufs=2, space="PSUM") as ps11,
                tc.tile_pool(name="ps12", bufs=5, space="PSUM") as ps12,
                tc.tile_pool(name="ps22", bufs=1, space="PSUM") as ps22,
            ):
                for cidx, (s0, ln) in enumerate(chunks):
                    p11 = conv14(ps11, "p11", s0, ln, wsb["w11i"],
                                 wsb["w11gp"], wsb["w11gw"], wsb["w11gs"])
                    w11oc = convp.tile([128, NCHUNK], f16, tag="w11oc")
                    nc.scalar.activation(w11oc[:, :ln], p11[:, :ln],
                                         AF.Relu, bias=wsb["b11"][:, 0:1])

                    def xop(t):
                        di, dj = taps[t]
                        xoff = PAD + s0 + di * WP + dj - 1
                        if xoff % 2 == 0:
                            return xs[:, xoff:xoff + ln]
                        return xs2[:, xoff - 1:xoff - 1 + ln]

                    p12s = {}
                    w12ocs = {}
                    for t in range(9):
                        p12 = ps12.tile([128, NCHUNK], f32, tag="p12")
                        nc.tensor.matmul(p12[:, :ln], wsb["w12T"][:, t, :],
                                         w11oc[:, :ln], start=True, stop=True)
                        p12s[t] = p12
                        if t <= 6:  # D on scalar
                            w12oc = w12p.tile([128, NCHUNK], f16, tag="w12oc")
                            nc.scalar.activation(
                                w12oc[:, :ln], p12[:, :ln], AF.Identity,
                                bias=wsb["b12T"][:, t:t + 1])
                            w12ocs[t] = w12oc

                    # vector stream: muls 0-4, D7, D8, mul7, mul8, v-chain
                    accv = prodp.tile([128, NCHUNK], f16, tag="accv")
                    pv = prodp.tile([128, NCHUNK], f16, tag="pv")
                    nc.vector.tensor_mul(accv[:, :ln], w12ocs[0][:, :ln], xop(0))
                    nc.vector.tensor_mul(pv[:, :ln], w12ocs[1][:, :ln], xop(1))
                    nc.vector.tensor_add(accv[:, :ln], accv[:, :ln], pv[:, :ln])
                    for t in (2, 3, 4):
                        pvt = prodp.tile([128, NCHUNK], f16, tag="pv")
                        nc.vector.tensor_mul(pvt[:, :ln], w12ocs[t][:, :ln],
                                             xop(t))
                        nc.vector.tensor_add(accv[:, :ln], accv[:, :ln],
                                             pvt[:, :ln])
                    # gpsimd stream: muls 5-6 + g-chain
                    pg5 = prodp.tile([128, NCHUNK], f16, tag="pg5")
                    pg6 = prodp.tile([128, NCHUNK], f16, tag="pg6")
                    accg = prodp.tile([128, NCHUNK], f16, tag="accg")
                    nc.gpsimd.tensor_mul(pg5[:, :ln], w12ocs[5][:, :ln], xop(5))
                    nc.gpsimd.tensor_mul(pg6[:, :ln], w12ocs[6][:, :ln], xop(6))
                    nc.gpsimd.tensor_add(accg[:, :ln], pg5[:, :ln], pg6[:, :ln])
                    # taps 7, 8: vector downcast from PSUM then mul
                    for t in (7, 8):
                        w12oc = w12p.tile([128, NCHUNK], f16, tag="w12ocv")
                        nc.vector.tensor_scalar_add(w12oc[:, :ln],
                                                    p12s[t][:, :ln],
                                                    wsb["b12T"][:, t:t + 1])
                        pvt = prodp.tile([128, NCHUNK], f16, tag="pv")
                        nc.vector.tensor_mul(pvt[:, :ln], w12oc[:, :ln], xop(t))
                        nc.vector.tensor_add(accv[:, :ln], accv[:, :ln],
                                             pvt[:, :ln])
                    nc.vector.tensor_add(accv[:, :ln], accv[:, :ln],
                                         accg[:, :ln])
                    nc.scalar.activation(localp[:, s0:s0 + ln], accv[:, :ln],
                                         AF.Relu, bias=wsb["bias1"][:, 0:1])

                    # interleaved conv22 half (16 matvec passes, chunks 6-13)
                    if 6 <= cidx <= 13:
                        for k in range(2):
                            idx = (cidx - 6) * 2 + k
                            wc = wcp.tile([128, NCHUNK], f16, tag="w22c")
                            nc.sync.dma_start(
                                wc[:],
                                w22hd.ap()[:, NCHUNK * idx:NCHUNK * (idx + 1)])
                            p22 = ps22.tile([1, NCHUNK], f32, tag="p22")
                            nc.tensor.matmul(p22[:], pool_sum[:, 0:1], wc[:],
                                             start=True, stop=True)
                            p22s = wcp.tile([1, NCHUNK], f32, tag="p22s")
                            if k == 0:
                                nc.scalar.copy(p22s[:], p22[:])
                            else:
                                nc.vector.tensor_copy(p22s[:], p22[:])
                            nc.sync.dma_start(
                                w22stage[4 * idx:4 * idx + 4, :],
                                p22s[0:1, :].rearrange("o (a b) -> o a b", a=4))

            # exchange conv22 halves: [64,128] + [64,128] -> [128,128]
            nc.gpsimd.collective_compute(
                "AllGather", mybir.AluOpType.bypass,
                replica_groups=[[0, 1], [2, 3], [4, 5], [6, 7]],
                ins=[w22stage.opt()], outs=[w22full.opt()],
            )
            w22of = work.tile([128, 128], f32, tag="w22of")
            nc.sync.dma_start(w22of[:], w22full[:])
            nc.vector.tensor_add(w22of[:], w22of[:], wsb["b22T"][:, :])
            w22oT = work.tile([128, 128], f16, tag="w22oT")
            nc.vector.tensor_copy(w22oT[:], w22of[:])

            # ============ phases 4+5: bmm -> BN2 -> conv3 -> BN3, 1-chunk lag
            nc.vector.memset(bn2o[:, 0:PADB], 0.0)
            nc.vector.memset(bn2o[:, PADB + NF_A:], 0.0)
            rowsv = bn2o[:, PADB:PADB + NF_A].rearrange("p (r c) -> p r c",
                                                        c=WP)

            with (
                tc.tile_pool(name="psb", bufs=2, space="PSUM") as psb,
                tc.tile_pool(name="ps3", bufs=2, space="PSUM") as ps3,
            ):
                def do_bmm(c):
                    s0, ln = chunks[c]
                    pb = psb.tile([128, NCHUNK], f32, tag="pb")
                    nc.tensor.matmul(pb[:, :ln], w22oT[:, :],
                                     localp[:, s0:s0 + ln], start=True,
                                     stop=True)
                    nc.scalar.activation(bn2o[:, PADB + s0:PADB + s0 + ln],
                                         pb[:, :ln], AF.Relu,
                                         bias=wsb["bias2"][:, 0:1])
                    # zero the wrap-around pad cols inside this chunk's range
                    r0 = -(-s0 // WP)
                    r1 = -(-(s0 + ln) // WP)
                    if r1 > r0:
                        nc.gpsimd.memset(rowsv[:, r0:r1, 0:1], 0.0)
                    rr0 = -(-(s0 - (WP - 1)) // WP)
                    rr1 = -(-(s0 + ln - (WP - 1)) // WP)
                    if rr1 > rr0:
                        nc.gpsimd.memset(rowsv[:, rr0:rr1, WP - 1:WP], 0.0)
                    if c == 0:  # A-row 0: outside-image halo for top half
                        rv = bn2o[:, PADB:PADB + WP]
                        nc.vector.tensor_mul(
                            rv, rv, msk[:, 0:1].to_broadcast((128, WP)))
                    if c == len(chunks) - 1:  # A-row 65
                        rv = bn2o[:, PADB + (AROWS - 1) * WP:PADB + NF_A]
                        nc.vector.tensor_mul(
                            rv, rv, msk[:, 1:2].to_broadcast((128, WP)))

                def do_conv3(c):
                    s0, ln = chunks[c]
                    p3 = ps3.tile([128, NCHUNK], f32, tag="p3")
                    for t, (di, dj) in enumerate(taps):
                        off = PADB + s0 + (di - 1) * WP + (dj - 1)
                        nc.tensor.matmul(p3[:, :ln], wsb["w3T"][:, t, :],
                                         bn2o[:, off:off + ln],
                                         start=(t == 0), stop=(t == 8))
                    nc.scalar.activation(outsl[:, s0:s0 + ln], p3[:, :ln],
                                         AF.Relu, bias=wsb["bias3"][:, 0:1])

                # store owned rows [a, b) of the half-image as they complete
                ovr = outsl[:].rearrange("p (r c) -> p r c", c=WP)

                def store(a, b):
                    nc.sync.dma_start(outd.ap()[:, a:b, :],
                                      ovr[:, 1 + a:1 + b, 1:1 + W])

                stores = {5: (0, 21), 9: (21, 37), 13: (37, 53)}
                do_bmm(0)
                for c in range(1, len(chunks)):
                    do_bmm(c)
                    do_conv3(c - 1)
                    if (c - 1) in stores:
                        store(*stores[c - 1])
                do_conv3(len(chunks) - 1)
                store(53, HALF)

    nc.compile()
    return nc


def _get_nc():
    if "nc" not in _CACHE:
        _CACHE["nc"] = _build_nc()
    return _CACHE["nc"]


def kernel(**inputs) -> np.ndarray:
    from concourse.bass_utils import run_bass_kernel_spmd

    nc = _get_nc()
    params, w22halves = _prep_params(
        **{k: np.asarray(v) for k, v in inputs.items()
           if k not in ("input", "weight")})
    slabs = _prep_slabs(np.asarray(inputs["input"], np.float32),
                        np.asarray(inputs["weight"], np.float32))
    in_maps = []
    for core in range(N_CORES):
        xflat, x2, gpa, gww, mask = slabs[core]
        m = {"x": xflat, "x2": x2, "gpa": gpa, "gww": gww, "mask": mask,
             "w22half": w22halves[core % 2]}
        m.update(params)
        in_maps.append(m)
    res = run_bass_kernel_spmd(nc, in_maps, core_ids=list(range(N_CORES)),
                               **_CACHE.get("run_kwargs", {}))
    _CACHE["last_results"] = res
    out = np.empty((B, Ci, H, W), np.float32)
    for core in range(N_CORES):
        b, half = core // 2, core % 2
        out[b, :, half * HALF:(half + 1) * HALF, :] = \
            res.results[core]["out"].astype(np.float32)
    return out


# revision 12
# speedup vs baseline: 1.2361x; 1.1479x over previous
"""Trainium2 Bass kernel for the guided dynamic-conv CNN (nn_KernelLearningGuide).

Sharding: 8 cores = 4 images x 2 row-halves (64 rows each). Each core gets
host-padded fp16 slabs (68 rows x 130 cols incl. conv padding + halo rows) so
every 3x3 conv is a bank of matmuls over flat pixel slices at tap offsets.
Guidance taps are pair-packed into stacked 128-row slabs (G|G<<1 and G|G<<WP)
so each 3x3-over-192-channels conv is 14 matmuls (the K=64 guidance taps ride
two-per-pass). The per-pixel guided conv spreads its elementwise work across
scalar (PSUM downcast+bias), vector and gpsimd. conv22 (pooled matvec) is
split across the core pair and recombined with an AllGather; the only other
cross-core exchange is the [128] mean-pool AllReduce. All matmul operands are
fp16 (full PE rate), fp32 PSUM accumulation; BN scales folded host-side.
"""

import sys

if "/opt/trn_rl_repo" not in sys.path:
    sys.path.insert(0, "/opt/trn_rl_repo")

import numpy as np

# ---- problem constants (hardcoded per spec) ----
B, Ci, Cg, H, W = 4, 128, 64, 128, 128
N_CORES = 8
HALF = H // 2          # 64 rows per core
WP = W + 2             # 130 padded cols
SROWS = HALF + 4       # 68 slab rows (2 halo+pad rows each side)
AROWS = HALF + 2       # 66 stage-A rows (output rows + 1 halo row each side)
PAD = 2                # extra flat elements at each slab end for tap overhang
NF_IN = SROWS * WP + 2 * PAD   # 8844 input slab flat length
NF_A = AROWS * WP              # 8580 stage-A flat length
PADB = WP + 1                  # 131 bn2out flat pad each side (conv3 overhang)
NCHUNK = 512
EPS = 1e-5

# phase-1 (conv21) only needs the 64 owned rows: flat [130, 8450)
P1_BASE = WP
P1_LEN = HALF * WP             # 8320

_CACHE = {}


def _taps():
    return [(di, dj) for di in range(3) for dj in range(3)]


def _prep_params(w11, b11, w12, b12, w21, b21, w22, b22,
                 g1, be1, m1, v1, g2, be2, m2, v2, w3, g3, be3, m3, v3):
    """Host-side parameter re-layouts + BN folding. Returns dict of np arrays
    (w22half is a 2-tuple: even-core half / odd-core half)."""
    f16 = np.float16
    s1 = (g1 / np.sqrt(v1 + EPS)).astype(np.float64)
    bias1 = (be1 - m1 * g1 / np.sqrt(v1 + EPS)).astype(np.float32)
    s2 = (g2 / np.sqrt(v2 + EPS)).astype(np.float64)
    bias2 = (be2 - m2 * g2 / np.sqrt(v2 + EPS)).astype(np.float32)
    s3 = (g3 / np.sqrt(v3 + EPS)).astype(np.float64)
    bias3 = (be3 - m3 * g3 / np.sqrt(v3 + EPS)).astype(np.float32)

    def conv3x3_lhst(w):  # w [Mout, K, 3, 3] -> [K, 9, Mout]
        return np.ascontiguousarray(np.transpose(w, (1, 2, 3, 0)).reshape(
            w.shape[1], 9, w.shape[0]))

    # conv11 / conv21: input part (K=128, 9 taps) and guidance part (K=64,
    # pair-packed into stacked-128 operands):
    #   wgp[p]: taps (p,0) top / (p,1) bottom   (rhs = gpa at d(p,0))
    #   wgw:    taps (0,2) top / (1,2) bottom   (rhs = gww at d(0,2))
    #   wgs:    tap (2,2) top / zero bottom     (rhs = gpa at d(2,2))
    def split_conv(w):
        wi = conv3x3_lhst(w[:, :Ci])                  # [128, 9, 128]
        wg = conv3x3_lhst(w[:, Ci:])                  # [64, 9, 128]
        wgp = np.zeros((128, 3, 128), np.float32)
        for p in range(3):
            wgp[:64, p] = wg[:, 3 * p]
            wgp[64:, p] = wg[:, 3 * p + 1]
        wgw = np.zeros((128, 1, 128), np.float32)
        wgw[:64, 0] = wg[:, 2]
        wgw[64:, 0] = wg[:, 5]
        wgs = np.zeros((128, 1, 128), np.float32)
        wgs[:64, 0] = wg[:, 8]
        return wi, wgp, wgw, wgs

    w11i, w11gp, w11gw, w11gs = split_conv(w11)
    w21i, w21gp, w21gw, w21gs = split_conv(w21)

    # conv12 (1x1): w12 [Ci*9, Ci] -> lhsT [K=128, 9, 128] with BN1 scale folded
    w12m = w12.reshape(Ci, 9, Ci).astype(np.float64)      # [c, t, k]
    w12m = w12m * s1[:, None, None]
    w12T = np.ascontiguousarray(np.transpose(w12m, (2, 1, 0)))  # [k, t, c]
    b12T = np.ascontiguousarray(
        (b12.reshape(Ci, 9).astype(np.float64) * s1[:, None]).astype(np.float32))

    # conv22: w22 [Ci*Ci, Ci]; fold /(H*W) mean and BN2 scale s2 (per out-ch i)
    w22m = w22.reshape(Ci, Ci, Ci).astype(np.float64)     # [i, j, k]
    w22m = w22m * (s2[:, None, None] / (H * W))
    w22pp = np.transpose(w22m, (2, 1, 0)).reshape(Ci, Ci * Ci)  # [k, (j,i)]
    w22halves = (np.ascontiguousarray(w22pp[:, :Ci * Ci // 2]).astype(f16),
                 np.ascontiguousarray(w22pp[:, Ci * Ci // 2:]).astype(f16))
    b22T = np.ascontiguousarray(
        (b22.reshape(Ci, Ci).astype(np.float64) * s2[:, None]).T.astype(np.float32))
    # b22T[j, i] = b22[i*Ci+j] * s2[i]

    # conv3: fold BN3 scale s3 per out-channel m
    w3m = w3.astype(np.float64) * s3[:, None, None, None]
    w3T = conv3x3_lhst(w3m)                               # [128, 9, 128]

    return dict(
        w11i=w11i.astype(f16), w11gp=w11gp.astype(f16),
        w11gw=w11gw.astype(f16), w11gs=w11gs.astype(f16),
        w21i=w21i.astype(f16), w21gp=w21gp.astype(f16),
        w21gw=w21gw.astype(f16), w21gs=w21gs.astype(f16),
        w12T=w12T.astype(f16), b12T=b12T,
        b22T=b22T,
        w3T=w3T.astype(f16),
        b11=np.ascontiguousarray(b11.astype(np.float32)[:, None]),
        b21=np.ascontiguousarray(b21.astype(np.float32)[:, None]),
        bias1=np.ascontiguousarray(bias1[:, None]),
        bias2=np.ascontiguousarray(bias2[:, None]),
        bias3=np.ascontiguousarray(bias3[:, None]),
    ), w22halves


def _prep_slabs(input, weight):
    """Per-core fp16 slabs: xs, xs2 (=xs<<1), gpa (G|G<<1), gww (G|G<<WP)."""
    f16 = np.float16
    xp = np.pad(input, ((0, 0), (0, 0), (2, 2), (1, 1)))    # [B, Ci, 132, 130]
    gp = np.pad(weight, ((0, 0), (0, 0), (2, 2), (1, 1)))   # [B, Cg, 132, 130]
    slabs = []
    for core in range(N_CORES):
        b, half = core // 2, core % 2
        r0 = half * HALF
        xsrow = xp[b, :, r0:r0 + SROWS].reshape(Ci, -1).astype(f16)
        gsrow = gp[b, :, r0:r0 + SROWS].reshape(Cg, -1).astype(f16)
        xflat = np.zeros((Ci, NF_IN), f16)
        xflat[:, PAD:PAD + SROWS * WP] = xsrow
        x2 = np.zeros((Ci, NF_IN), f16)
        x2[:, :-1] = xflat[:, 1:]
        gflat = np.zeros((Cg, NF_IN), f16)
        gflat[:, PAD:PAD + SROWS * WP] = gsrow
        gpa = np.zeros((128, NF_IN), f16)
        gpa[:64] = gflat
        gpa[64:, :-1] = gflat[:, 1:]
        gww = np.zeros((128, NF_IN), f16)
        gww[:64] = gflat
        gww[64:, :-WP] = gflat[:, WP:]
        mask = np.zeros((128, 2), np.float32)
        mask[:, 0] = 0.0 if half == 0 else 1.0   # A-row 0 (image row r0-1)
        mask[:, 1] = 1.0 if half == 0 else 0.0   # A-row 65 (image row r0+64)
        slabs.append((np.ascontiguousarray(xflat), np.ascontiguousarray(x2),
                      np.ascontiguousarray(gpa), np.ascontiguousarray(gww),
                      mask))
    return slabs


def _build_nc():
    import concourse.bass as bass
    import concourse.mybir as mybir
    import concourse.tile as tile
    from concourse import bacc

    f16, f32 = mybir.dt.float16, mybir.dt.float32
    AF = mybir.ActivationFunctionType
    nc = bacc.Bacc("TRN2", target_bir_lowering=False, debug=False,
                   num_devices=N_CORES)

    # ---- DRAM I/O ----
    xd = nc.dram_tensor("x", [Ci, NF_IN], f16, kind="ExternalInput")
    x2d = nc.dram_tensor("x2", [Ci, NF_IN], f16, kind="ExternalInput")
    gpad = nc.dram_tensor("gpa", [128, NF_IN], f16, kind="ExternalInput")
    gwwd = nc.dram_tensor("gww", [128, NF_IN], f16, kind="ExternalInput")
    maskd = nc.dram_tensor("mask", [128, 2], f32, kind="ExternalInput")
    wd = {}
    for name, shape, dt in (
        ("w11i", [128, 9, 128], f16), ("w11gp", [128, 3, 128], f16),
        ("w11gw", [128, 1, 128], f16), ("w11gs", [128, 1, 128], f16),
        ("w21i", [128, 9, 128], f16), ("w21gp", [128, 3, 128], f16),
        ("w21gw", [128, 1, 128], f16), ("w21gs", [128, 1, 128], f16),
        ("w12T", [128, 9, 128], f16), ("b12T", [128, 9], f32),
        ("b22T", [128, 128], f32),
        ("w3T", [128, 9, 128], f16),
        ("b11", [128, 1], f32), ("b21", [128, 1], f32),
        ("bias1", [128, 1], f32), ("bias2", [128, 1], f32),
        ("bias3", [128, 1], f32),
    ):
        wd[name] = nc.dram_tensor(name, shape, dt, kind="ExternalInput")
    w22hd = nc.dram_tensor("w22half", [128, Ci * Ci // 2], f16,
                           kind="ExternalInput")
    outd = nc.dram_tensor("out", [Ci, HALF, W], f16, kind="ExternalOutput")

    taps = _taps()
    chunks = [(s0, min(NCHUNK, NF_A - s0)) for s0 in range(0, NF_A, NCHUNK)]
    p1chunks = [(P1_BASE + s, min(NCHUNK, P1_LEN - s))
                for s in range(0, P1_LEN, NCHUNK)]

    with tile.TileContext(nc) as tc:
        with (
            tc.tile_pool(name="wpool", bufs=1) as wpool,
            tc.tile_pool(name="slab", bufs=1) as slab,
            tc.tile_pool(name="convp", bufs=3) as convp,
            tc.tile_pool(name="w12p", bufs=4) as w12p,
            tc.tile_pool(name="prodp", bufs=3) as prodp,
            tc.tile_pool(name="work", bufs=2) as work,
            tc.tile_pool(name="wcp", bufs=4) as wcp,
            tc.tile_pool(name="dram", bufs=1, space="DRAM") as dram,
        ):
            # ---- weights + slabs: phase-1-critical DMAs first on the sync
            # queue; everything phase-2+ goes on the vector DGE queue so the
            # sync sequencer reaches the first slab pieces fast.
            wsb = {}
            p1_names = ("w21i", "w21gp", "w21gw", "w21gs", "b21")
            for name in p1_names:
                t = wd[name]
                wt = wpool.tile(list(t.shape), t.dtype, tag=name)
                nc.sync.dma_start(wt[:], t.ap())
                wsb[name] = wt

            xs = slab.tile([128, NF_IN], f16, tag="xs")
            xs2 = slab.tile([128, NF_IN], f16, tag="xs2")
            gpa = slab.tile([128, NF_IN], f16, tag="gpa")
            gww = slab.tile([128, NF_IN], f16, tag="gww")
            pieces = [0, 1280, 3840, 6400, NF_IN]
            for i in range(len(pieces) - 1):
                a, b = pieces[i], pieces[i + 1]
                for dst, src in ((xs, xd), (gpa, gpad), (gww, gwwd)):
                    nc.sync.dma_start(dst[:, a:b], src.ap()[:, a:b])
            for name, t in wd.items():
                if name in p1_names:
                    continue
                wt = wpool.tile(list(t.shape), t.dtype, tag=name)
                nc.scalar.dma_start(wt[:], t.ap())
                wsb[name] = wt
            msk = wpool.tile([128, 2], f32, tag="msk")
            nc.gpsimd.dma_start(msk[:], maskd.ap())
            for i in range(len(pieces) - 1):
                a, b = pieces[i], pieces[i + 1]
                nc.gpsimd.dma_start(xs2[:, a:b], x2d.ap()[:, a:b])

            # stage-A big fp16 slabs
            w21o = slab.tile([128, P1_BASE + P1_LEN], f16, tag="w21o")
            localp = slab.tile([128, NF_A], f16, tag="localp")
            bn2o = slab.tile([128, PADB + NF_A + PADB], f16, tag="bn2o")
            outsl = slab.tile([128, NF_A], f16, tag="outsl")

            def conv14(psum_pool, tag, s0, ln, wi, wgp, wgw, wgs):
                """3x3 conv over cat(x,g): 9 input taps + 5 packed guidance."""
                p = psum_pool.tile([128, NCHUNK], f32, tag=tag)
                for t, (di, dj) in enumerate(taps):
                    off = PAD + s0 + di * WP + dj - 1
                    nc.tensor.matmul(p[:, :ln], wi[:, t, :],
                                     xs[:, off:off + ln],
                                     start=(t == 0), stop=False)
                for pr in range(3):
                    off = PAD + s0 + pr * WP - 1
                    nc.tensor.matmul(p[:, :ln], wgp[:, pr, :],
                                     gpa[:, off:off + ln],
                                     start=False, stop=False)
                off = PAD + s0 + 1
                nc.tensor.matmul(p[:, :ln], wgw[:, 0, :],
                                 gww[:, off:off + ln], start=False, stop=False)
                off = PAD + s0 + 2 * WP + 1
                nc.tensor.matmul(p[:, :ln], wgs[:, 0, :],
                                 gpa[:, off:off + ln], start=False, stop=True)
                return p

            # ================= phase 1: conv21 -> w21o (owned rows) =========
            with tc.tile_pool(name="ps21", bufs=2, space="PSUM") as ps21:
                for s0, ln in p1chunks:
                    p = conv14(ps21, "p21", s0, ln, wsb["w21i"], wsb["w21gp"],
                               wsb["w21gw"], wsb["w21gs"])
                    nc.scalar.activation(w21o[:, s0:s0 + ln], p[:, :ln],
                                         AF.Relu, bias=wsb["b21"][:, 0:1])

            # ---- pool (owned 64 rows x 128 cols) + pairwise AllReduce ----
            pool_part = work.tile([128, 1], f32, tag="poolp")
            own = w21o[:].rearrange("p (r c) -> p r c", c=WP)[:, 1:1 + HALF,
                                                             1:1 + W]
            nc.vector.reduce_sum(pool_part[:, 0:1], own,
                                 axis=mybir.AxisListType.XY)
            cin = dram.tile([128, 1], f32)
            cout = dram.tile([128, 1], f32)
            nc.gpsimd.dma_start(cin[:], pool_part[:])
            nc.gpsimd.collective_compute(
                "AllReduce", mybir.AluOpType.add,
                replica_groups=[[0, 1], [2, 3], [4, 5], [6, 7]],
                ins=[cin.opt()], outs=[cout.opt()],
            )
            pool_sum = work.tile([128, 1], f16, tag="pools")
            pool_f32 = work.tile([128, 1], f32, tag="poolf")
            nc.gpsimd.dma_start(pool_f32[:], cout[:])
            nc.gpsimd.tensor_copy(pool_sum[:], pool_f32[:])

            # conv22 half result staging (this core computes 64 of 128 j-rows)
            w22stage = dram.tile([64, 128], f32)
            w22full = dram.tile([128, 128], f32)

            # ============ phase 2: conv11 -> conv12 -> guided -> BN1 ========
            # per tap: D = PSUM downcast+bias (scalar for taps 0-6, vector for
            # 7-8), M = product with shifted x (vector; taps 5-6 on gpsimd),
            # two accumulate chains (vector / gpsimd) joined at the end.
            with (
                tc.tile_pool(name="ps11", bufs=2, space="PSUM") as ps11,
                tc.tile_pool(name="ps12", bufs=5, space="PSUM") as ps12,
                tc.tile_pool(name="ps22", bufs=1, space="PSUM") as ps22,
            ):
                for cidx, (s0, ln) in enumerate(chunks):
                    p11 = conv14(ps11, "p11", s0, ln, wsb["w11i"],
                                 wsb["w11gp"], wsb["w11gw"], wsb["w11gs"])
                    w11oc = convp.tile([128, NCHUNK], f16, tag="w11oc")
                    nc.scalar.activation(w11oc[:, :ln], p11[:, :ln],
                                         AF.Relu, bias=wsb["b11"][:, 0:1])

                    def xop(t):
                        di, dj = taps[t]
                        xoff = PAD + s0 + di * WP + dj - 1
                        if xoff % 2 == 0:
                            return xs[:, xoff:xoff + ln]
                        return xs2[:, xoff - 1:xoff - 1 + ln]

                    p12s = {}
                    w12ocs = {}
                    for t in range(9):
                        p12 = ps12.tile([128, NCHUNK], f32, tag="p12")
                        nc.tensor.matmul(p12[:, :ln], wsb["w12T"][:, t, :],
                                         w11oc[:, :ln], start=True, stop=True)
                        p12s[t] = p12
                        if t <= 7:  # D (downcast + bias) on scalar
                            w12oc = w12p.tile([128, NCHUNK], f16, tag="w12oc")
                            nc.scalar.activation(
                                w12oc[:, :ln], p12[:, :ln], AF.Identity,
                                bias=wsb["b12T"][:, t:t + 1])
                            w12ocs[t] = w12oc

                    # vector stream: 8 muls + fused tap 8 + 8-add chain
                    accv = prodp.tile([128, NCHUNK], f16, tag="accv")
                    nc.vector.tensor_mul(accv[:, :ln], w12ocs[0][:, :ln], xop(0))
                    for t in range(1, 8):
                        pvt = prodp.tile([128, NCHUNK], f16, tag="pv")
                        nc.vector.tensor_mul(pvt[:, :ln], w12ocs[t][:, :ln],
                                             xop(t))
                        nc.vector.tensor_add(accv[:, :ln], accv[:, :ln],
                                             pvt[:, :ln])
                    pv8 = prodp.tile([128, NCHUNK], f16, tag="pv")
                    nc.vector.scalar_tensor_tensor(
                        pv8[:, :ln], p12s[8][:, :ln], wsb["b12T"][:, 8:9],
                        xop(8), op0=mybir.AluOpType.add,
                        op1=mybir.AluOpType.mult)
                    nc.vector.tensor_add(accv[:, :ln], accv[:, :ln],
                                         pv8[:, :ln])
                    nc.scalar.activation(localp[:, s0:s0 + ln], accv[:, :ln],
                                         AF.Relu, bias=wsb["bias1"][:, 0:1])

                    # interleaved conv22 half (16 matvec passes, chunks 6-13)
                    if 6 <= cidx <= 13:
                        for k in range(2):
                            idx = (cidx - 6) * 2 + k
                            wc = wcp.tile([128, NCHUNK], f16, tag="w22c")
                            nc.sync.dma_start(
                                wc[:],
                                w22hd.ap()[:, NCHUNK * idx:NCHUNK * (idx + 1)])
                            p22 = ps22.tile([1, NCHUNK], f32, tag="p22")
                            nc.tensor.matmul(p22[:], pool_sum[:, 0:1], wc[:],
                                             start=True, stop=True)
                            p22s = wcp.tile([1, NCHUNK], f32, tag="p22s")
                            if k == 0:
                                nc.scalar.copy(p22s[:], p22[:])
                            else:
                                nc.vector.tensor_copy(p22s[:], p22[:])
                            nc.sync.dma_start(
                                w22stage[4 * idx:4 * idx + 4, :],
                                p22s[0:1, :].rearrange("o (a b) -> o a b", a=4))
                    if cidx == 13:
                        # exchange halves: [64,128]+[64,128] -> [128,128];
                        # finalize on gpsimd (idle; keeps vector/scalar clear)
                        nc.gpsimd.collective_compute(
                            "AllGather", mybir.AluOpType.bypass,
                            replica_groups=[[0, 1], [2, 3], [4, 5], [6, 7]],
                            ins=[w22stage.opt()], outs=[w22full.opt()],
                        )
                        w22of = work.tile([128, 128], f32, tag="w22of")
                        nc.gpsimd.dma_start(w22of[:], w22full[:])
                        nc.gpsimd.tensor_add(w22of[:], w22of[:],
                                             wsb["b22T"][:, :])
                        w22oT = work.tile([128, 128], f16, tag="w22oT")
                        nc.gpsimd.tensor_copy(w22oT[:], w22of[:])

            # ============ phases 4+5: bmm -> BN2 -> conv3 -> BN3, 1-chunk lag
            nc.gpsimd.memset(bn2o[:, 0:PADB], 0.0)
            nc.gpsimd.memset(bn2o[:, PADB + NF_A:], 0.0)
            rowsv = bn2o[:, PADB:PADB + NF_A].rearrange("p (r c) -> p r c",
                                                        c=WP)

            with (
                tc.tile_pool(name="psb", bufs=2, space="PSUM") as psb,
                tc.tile_pool(name="ps3", bufs=2, space="PSUM") as ps3,
            ):
                def do_bmm(c):
                    s0, ln = chunks[c]
                    pb = psb.tile([128, NCHUNK], f32, tag="pb")
                    nc.tensor.matmul(pb[:, :ln], w22oT[:, :],
                                     localp[:, s0:s0 + ln], start=True,
                                     stop=True)
                    nc.scalar.activation(bn2o[:, PADB + s0:PADB + s0 + ln],
                                         pb[:, :ln], AF.Relu,
                                         bias=wsb["bias2"][:, 0:1])
                    # zero the wrap-around pad cols inside this chunk's range
                    r0 = -(-s0 // WP)
                    r1 = -(-(s0 + ln) // WP)
                    if r1 > r0:
                        nc.gpsimd.memset(rowsv[:, r0:r1, 0:1], 0.0)
                    rr0 = -(-(s0 - (WP - 1)) // WP)
                    rr1 = -(-(s0 + ln - (WP - 1)) // WP)
                    if rr1 > rr0:
                        nc.gpsimd.memset(rowsv[:, rr0:rr1, WP - 1:WP], 0.0)
                    if c == 0:  # A-row 0: outside-image halo for top half
                        rv = bn2o[:, PADB:PADB + WP]
                        nc.gpsimd.tensor_mul(
                            rv, rv, msk[:, 0:1].to_broadcast((128, WP)))
                    if c == len(chunks) - 1:  # A-row 65
                        rv = bn2o[:, PADB + (AROWS - 1) * WP:PADB + NF_A]
                        nc.gpsimd.tensor_mul(
                            rv, rv, msk[:, 1:2].to_broadcast((128, WP)))

                def do_conv3(c):
                    s0, ln = chunks[c]
                    p3 = ps3.tile([128, NCHUNK], f32, tag="p3")
                    for t, (di, dj) in enumerate(taps):
                        off = PADB + s0 + (di - 1) * WP + (dj - 1)
                        nc.tensor.matmul(p3[:, :ln], wsb["w3T"][:, t, :],
                                         bn2o[:, off:off + ln],
                                         start=(t == 0), stop=(t == 8))
                    nc.scalar.activation(outsl[:, s0:s0 + ln], p3[:, :ln],
                                         AF.Relu, bias=wsb["bias3"][:, 0:1])

                # store owned rows [a, b) of the half-image as they complete
                ovr = outsl[:].rearrange("p (r c) -> p r c", c=WP)

                def store(a, b):
                    nc.sync.dma_start(outd.ap()[:, a:b, :],
                                      ovr[:, 1 + a:1 + b, 1:1 + W])

                stores = {5: (0, 21), 9: (21, 37), 13: (37, 53)}
                do_bmm(0)
                for c in range(1, len(chunks)):
                    do_bmm(c)
                    do_conv3(c - 1)
                    if (c - 1) in stores:
                        store(*stores[c - 1])
                do_conv3(len(chunks) - 1)
                store(53, HALF)

    nc.compile()
    return nc


def _get_nc():
    if "nc" not in _CACHE:
        _CACHE["nc"] = _build_nc()
    return _CACHE["nc"]


def kernel(**inputs) -> np.ndarray:
    from concourse.bass_utils import run_bass_kernel_spmd

    nc = _get_nc()
    params, w22halves = _prep_params(
        **{k: np.asarray(v) for k, v in inputs.items()
           if k not in ("input", "weight")})
    slabs = _prep_slabs(np.asarray(inputs["input"], np.float32),
                        np.asarray(inputs["weight"], np.float32))
    in_maps = []
    for core in range(N_CORES):
        xflat, x2, gpa, gww, mask = slabs[core]
        m = {"x": xflat, "x2": x2, "gpa": gpa, "gww": gww, "mask": mask,
             "w22half": w22halves[core % 2]}
        m.update(params)
        in_maps.append(m)
    res = run_bass_kernel_spmd(nc, in_maps, core_ids=list(range(N_CORES)),
                               **_CACHE.get("run_kwargs", {}))
    _CACHE["last_results"] = res
    out = np.empty((B, Ci, H, W), np.float32)
    for core in range(N_CORES):
        b, half = core // 2, core % 2
        out[b, :, half * HALF:(half + 1) * HALF, :] = \
            res.results[core]["out"].astype(np.float32)
    return out


# revision 18
# speedup vs baseline: 1.3328x; 1.0782x over previous
"""Trainium2 Bass kernel for the guided dynamic-conv CNN (nn_KernelLearningGuide).

Sharding: 8 cores = 4 images x 2 row-halves (64 rows each). Each core gets
host-padded fp16 slabs (68 rows x 130 cols incl. conv padding + halo rows) so
every 3x3 conv is a bank of matmuls over flat pixel slices at tap offsets.
Guidance taps are pair-packed into stacked 128-row slabs (G|G<<1 and G|G<<WP)
so each 3x3-over-192-channels conv is 14 matmuls (the K=64 guidance taps ride
two-per-pass). The per-pixel guided conv spreads its elementwise work across
scalar (PSUM downcast+bias), vector and gpsimd. conv22 (pooled matvec) is
split across the core pair and recombined with an AllGather; the only other
cross-core exchange is the [128] mean-pool AllReduce. All matmul operands are
fp16 (full PE rate), fp32 PSUM accumulation; BN scales folded host-side.
"""

import sys

if "/opt/trn_rl_repo" not in sys.path:
    sys.path.insert(0, "/opt/trn_rl_repo")

import numpy as np

# ---- problem constants (hardcoded per spec) ----
B, Ci, Cg, H, W = 4, 128, 64, 128, 128
N_CORES = 8
HALF = H // 2          # 64 rows per core
WP = W + 2             # 130 padded cols
SROWS = HALF + 4       # 68 slab rows (2 halo+pad rows each side)
AROWS = HALF + 2       # 66 stage-A rows (output rows + 1 halo row each side)
PAD = 2                # extra flat elements at each slab end for tap overhang
NF_IN = SROWS * WP + 2 * PAD   # 8844 input slab flat length
NF_A = AROWS * WP              # 8580 stage-A flat length
PADB = WP + 1                  # 131 bn2out flat pad each side (conv3 overhang)
NCHUNK = 512
EPS = 1e-5

# phase-1 (conv21) only needs the 64 owned rows: flat [130, 8450)
P1_BASE = WP
P1_LEN = HALF * WP             # 8320

_CACHE = {}


def _taps():
    return [(di, dj) for di in range(3) for dj in range(3)]


def _prep_params(w11, b11, w12, b12, w21, b21, w22, b22,
                 g1, be1, m1, v1, g2, be2, m2, v2, w3, g3, be3, m3, v3):
    """Host-side parameter re-layouts + BN folding. Returns dict of np arrays
    (w22half is a 2-tuple: even-core half / odd-core half)."""
    f16 = np.float16
    s1 = (g1 / np.sqrt(v1 + EPS)).astype(np.float64)
    bias1 = (be1 - m1 * g1 / np.sqrt(v1 + EPS)).astype(np.float32)
    s2 = (g2 / np.sqrt(v2 + EPS)).astype(np.float64)
    bias2 = (be2 - m2 * g2 / np.sqrt(v2 + EPS)).astype(np.float32)
    s3 = (g3 / np.sqrt(v3 + EPS)).astype(np.float64)
    bias3 = (be3 - m3 * g3 / np.sqrt(v3 + EPS)).astype(np.float32)

    def conv3x3_lhst(w):  # w [Mout, K, 3, 3] -> [K, 9, Mout]
        return np.ascontiguousarray(np.transpose(w, (1, 2, 3, 0)).reshape(
            w.shape[1], 9, w.shape[0]))

    # conv11 / conv21: input part (K=128, 9 taps) and guidance part (K=64,
    # pair-packed into stacked-128 operands):
    #   wgp[p]: taps (p,0) top / (p,1) bottom   (rhs = gpa at d(p,0))
    #   wgw:    taps (0,2) top / (1,2) bottom   (rhs = gww at d(0,2))
    #   wgs:    tap (2,2) top / zero bottom     (rhs = gpa at d(2,2))
    def split_conv(w):
        wi = conv3x3_lhst(w[:, :Ci])                  # [128, 9, 128]
        wg = conv3x3_lhst(w[:, Ci:])                  # [64, 9, 128]
        wgp = np.zeros((128, 3, 128), np.float32)
        for p in range(3):
            wgp[:64, p] = wg[:, 3 * p]
            wgp[64:, p] = wg[:, 3 * p + 1]
        wgw = np.zeros((128, 1, 128), np.float32)
        wgw[:64, 0] = wg[:, 2]
        wgw[64:, 0] = wg[:, 5]
        wgs = np.zeros((128, 1, 128), np.float32)
        wgs[:64, 0] = wg[:, 8]
        return wi, wgp, wgw, wgs

    w11i, w11gp, w11gw, w11gs = split_conv(w11)
    w21i, w21gp, w21gw, w21gs = split_conv(w21)

    # conv12 (1x1): w12 [Ci*9, Ci] -> lhsT [K=128, 9, 128] with BN1 scale folded
    w12m = w12.reshape(Ci, 9, Ci).astype(np.float64)      # [c, t, k]
    w12m = w12m * s1[:, None, None]
    w12T = np.ascontiguousarray(np.transpose(w12m, (2, 1, 0)))  # [k, t, c]
    b12T = np.ascontiguousarray(
        (b12.reshape(Ci, 9).astype(np.float64) * s1[:, None]).astype(np.float32))

    # conv22: w22 [Ci*Ci, Ci]; fold /(H*W) mean and BN2 scale s2 (per out-ch i)
    w22m = w22.reshape(Ci, Ci, Ci).astype(np.float64)     # [i, j, k]
    w22m = w22m * (s2[:, None, None] / (H * W))
    w22pp = np.transpose(w22m, (2, 1, 0)).reshape(Ci, Ci * Ci)  # [k, (j,i)]
    w22halves = (np.ascontiguousarray(w22pp[:, :Ci * Ci // 2]).astype(f16),
                 np.ascontiguousarray(w22pp[:, Ci * Ci // 2:]).astype(f16))
    b22T = np.ascontiguousarray(
        (b22.reshape(Ci, Ci).astype(np.float64) * s2[:, None]).T.astype(np.float32))
    # b22T[j, i] = b22[i*Ci+j] * s2[i]

    # conv3: fold BN3 scale s3 per out-channel m
    w3m = w3.astype(np.float64) * s3[:, None, None, None]
    w3T = conv3x3_lhst(w3m)                               # [128, 9, 128]

    return dict(
        w11i=w11i.astype(f16), w11gp=w11gp.astype(f16),
        w11gw=w11gw.astype(f16), w11gs=w11gs.astype(f16),
        w21i=w21i.astype(f16), w21gp=w21gp.astype(f16),
        w21gw=w21gw.astype(f16), w21gs=w21gs.astype(f16),
        w12T=w12T.astype(f16), b12T=b12T,
        b22T=b22T,
        w3T=w3T.astype(f16),
        b11=np.ascontiguousarray(b11.astype(np.float32)[:, None]),
        b21=np.ascontiguousarray(b21.astype(np.float32)[:, None]),
        bias1=np.ascontiguousarray(bias1[:, None]),
        bias2=np.ascontiguousarray(bias2[:, None]),
        bias3=np.ascontiguousarray(bias3[:, None]),
    ), w22halves


def _prep_slabs(input, weight):
    """Per-core fp16 slabs: xs, xs2 (=xs<<1), gpa (G|G<<1), gww (G|G<<WP)."""
    f16 = np.float16
    xp = np.pad(input, ((0, 0), (0, 0), (2, 2), (1, 1)))    # [B, Ci, 132, 130]
    gp = np.pad(weight, ((0, 0), (0, 0), (2, 2), (1, 1)))   # [B, Cg, 132, 130]
    slabs = []
    for core in range(N_CORES):
        b, half = core // 2, core % 2
        r0 = half * HALF
        xsrow = xp[b, :, r0:r0 + SROWS].reshape(Ci, -1).astype(f16)
        gsrow = gp[b, :, r0:r0 + SROWS].reshape(Cg, -1).astype(f16)
        xflat = np.zeros((Ci, NF_IN), f16)
        xflat[:, PAD:PAD + SROWS * WP] = xsrow
        x2 = np.zeros((Ci, NF_IN), f16)
        x2[:, :-1] = xflat[:, 1:]
        gflat = np.zeros((Cg, NF_IN), f16)
        gflat[:, PAD:PAD + SROWS * WP] = gsrow
        gpa = np.zeros((128, NF_IN), f16)
        gpa[:64] = gflat
        gpa[64:, :-1] = gflat[:, 1:]
        gww = np.zeros((128, NF_IN), f16)
        gww[:64] = gflat
        gww[64:, :-WP] = gflat[:, WP:]
        mask = np.zeros((128, 2), np.float32)
        mask[:, 0] = 0.0 if half == 0 else 1.0   # A-row 0 (image row r0-1)
        mask[:, 1] = 1.0 if half == 0 else 0.0   # A-row 65 (image row r0+64)
        slabs.append((np.ascontiguousarray(xflat), np.ascontiguousarray(x2),
                      np.ascontiguousarray(gpa), np.ascontiguousarray(gww),
                      mask))
    return slabs


def _build_nc():
    import concourse.bass as bass
    import concourse.mybir as mybir
    import concourse.tile as tile
    from concourse import bacc

    f16, f32 = mybir.dt.float16, mybir.dt.float32
    AF = mybir.ActivationFunctionType
    nc = bacc.Bacc("TRN2", target_bir_lowering=False, debug=False,
                   num_devices=N_CORES)

    # ---- DRAM I/O ----
    xd = nc.dram_tensor("x", [Ci, NF_IN], f16, kind="ExternalInput")
    x2d = nc.dram_tensor("x2", [Ci, NF_IN], f16, kind="ExternalInput")
    gpad = nc.dram_tensor("gpa", [128, NF_IN], f16, kind="ExternalInput")
    gwwd = nc.dram_tensor("gww", [128, NF_IN], f16, kind="ExternalInput")
    maskd = nc.dram_tensor("mask", [128, 2], f32, kind="ExternalInput")
    wd = {}
    for name, shape, dt in (
        ("w11i", [128, 9, 128], f16), ("w11gp", [128, 3, 128], f16),
        ("w11gw", [128, 1, 128], f16), ("w11gs", [128, 1, 128], f16),
        ("w21i", [128, 9, 128], f16), ("w21gp", [128, 3, 128], f16),
        ("w21gw", [128, 1, 128], f16), ("w21gs", [128, 1, 128], f16),
        ("w12T", [128, 9, 128], f16), ("b12T", [128, 9], f32),
        ("b22T", [128, 128], f32),
        ("w3T", [128, 9, 128], f16),
        ("b11", [128, 1], f32), ("b21", [128, 1], f32),
        ("bias1", [128, 1], f32), ("bias2", [128, 1], f32),
        ("bias3", [128, 1], f32),
    ):
        wd[name] = nc.dram_tensor(name, shape, dt, kind="ExternalInput")
    w22hd = nc.dram_tensor("w22half", [128, Ci * Ci // 2], f16,
                           kind="ExternalInput")
    outd = nc.dram_tensor("out", [Ci, HALF, W], f16, kind="ExternalOutput")

    taps = _taps()
    chunks = [(s0, min(NCHUNK, NF_A - s0)) for s0 in range(0, NF_A, NCHUNK)]
    p1chunks = [(P1_BASE + s, min(NCHUNK, P1_LEN - s))
                for s in range(0, P1_LEN, NCHUNK)]

    with tile.TileContext(nc) as tc:
        with (
            tc.tile_pool(name="wpool", bufs=1) as wpool,
            tc.tile_pool(name="slab", bufs=1) as slab,
            tc.tile_pool(name="convp", bufs=3) as convp,
            tc.tile_pool(name="w12p", bufs=4) as w12p,
            tc.tile_pool(name="prodp", bufs=2) as prodp,
            tc.tile_pool(name="work", bufs=2) as work,
            tc.tile_pool(name="wcp", bufs=4) as wcp,
            tc.tile_pool(name="dram", bufs=1, space="DRAM") as dram,
        ):
            # ---- weights + slabs: phase-1-critical DMAs first on the sync
            # queue; everything phase-2+ goes on the vector DGE queue so the
            # sync sequencer reaches the first slab pieces fast.
            wsb = {}
            p1_names = ("w21i", "w21gp", "w21gw", "w21gs", "b21")
            for name in p1_names:
                t = wd[name]
                wt = wpool.tile(list(t.shape), t.dtype, tag=name)
                nc.sync.dma_start(wt[:], t.ap())
                wsb[name] = wt

            xs = slab.tile([128, NF_IN], f16, tag="xs")
            xs2 = slab.tile([128, NF_IN], f16, tag="xs2")
            gpa = slab.tile([128, NF_IN], f16, tag="gpa")
            gww = slab.tile([128, NF_IN], f16, tag="gww")
            pieces = [0, 1280, 3840, 6400, NF_IN]
            for i in range(len(pieces) - 1):
                a, b = pieces[i], pieces[i + 1]
                for dst, src in ((xs, xd), (gpa, gpad), (gww, gwwd)):
                    nc.sync.dma_start(dst[:, a:b], src.ap()[:, a:b])
            for name, t in wd.items():
                if name in p1_names:
                    continue
                wt = wpool.tile(list(t.shape), t.dtype, tag=name)
                nc.scalar.dma_start(wt[:], t.ap())
                wsb[name] = wt
            msk = wpool.tile([128, 2], f32, tag="msk")
            nc.gpsimd.dma_start(msk[:], maskd.ap())
            for i in range(len(pieces) - 1):
                a, b = pieces[i], pieces[i + 1]
                nc.gpsimd.dma_start(xs2[:, a:b], x2d.ap()[:, a:b])

            # stage-A big fp16 slabs (outsl is allocated later, after the
            # w21o pool closes, so they can share SBUF space)
            localp = slab.tile([128, NF_A], f16, tag="localp")
            bn2o = slab.tile([128, PADB + NF_A + PADB], f16, tag="bn2o")

            def conv14(psum_pool, tag, s0, ln, wi, wgp, wgw, wgs):
                """3x3 conv over cat(x,g): 9 input taps + 5 packed guidance."""
                p = psum_pool.tile([128, NCHUNK], f32, tag=tag)
                for t, (di, dj) in enumerate(taps):
                    off = PAD + s0 + di * WP + dj - 1
                    nc.tensor.matmul(p[:, :ln], wi[:, t, :],
                                     xs[:, off:off + ln],
                                     start=(t == 0), stop=False)
                for pr in range(3):
                    off = PAD + s0 + pr * WP - 1
                    nc.tensor.matmul(p[:, :ln], wgp[:, pr, :],
                                     gpa[:, off:off + ln],
                                     start=False, stop=False)
                off = PAD + s0 + 1
                nc.tensor.matmul(p[:, :ln], wgw[:, 0, :],
                                 gww[:, off:off + ln], start=False, stop=False)
                off = PAD + s0 + 2 * WP + 1
                nc.tensor.matmul(p[:, :ln], wgs[:, 0, :],
                                 gpa[:, off:off + ln], start=False, stop=True)
                return p

            # ================= phase 1: conv21 -> w21o (owned rows) =========
            pool_part = work.tile([128, 1], f32, tag="poolp")
            with (
                tc.tile_pool(name="w21s", bufs=1) as w21s,
                tc.tile_pool(name="ps21", bufs=2, space="PSUM") as ps21,
            ):
                w21o = w21s.tile([128, P1_BASE + P1_LEN], f16, tag="w21o")
                for s0, ln in p1chunks:
                    p = conv14(ps21, "p21", s0, ln, wsb["w21i"], wsb["w21gp"],
                               wsb["w21gw"], wsb["w21gs"])
                    nc.scalar.activation(w21o[:, s0:s0 + ln], p[:, :ln],
                                         AF.Relu, bias=wsb["b21"][:, 0:1])

                # ---- pool (owned 64 rows x 128 cols) ----
                own = w21o[:].rearrange("p (r c) -> p r c", c=WP)[:, 1:1 + HALF,
                                                                 1:1 + W]
                nc.vector.reduce_sum(pool_part[:, 0:1], own,
                                     axis=mybir.AxisListType.XY)
            outsl = slab.tile([128, NF_A], f16, tag="outsl")

            # ---- pairwise AllReduce of the pool partial ----
            cin = dram.tile([128, 1], f32)
            cout = dram.tile([128, 1], f32)
            nc.gpsimd.dma_start(cin[:], pool_part[:])
            nc.gpsimd.collective_compute(
                "AllReduce", mybir.AluOpType.add,
                replica_groups=[[0, 1], [2, 3], [4, 5], [6, 7]],
                ins=[cin.opt()], outs=[cout.opt()],
            )
            pool_sum = work.tile([128, 1], f16, tag="pools")
            pool_f32 = work.tile([128, 1], f32, tag="poolf")
            nc.gpsimd.dma_start(pool_f32[:], cout[:])
            nc.gpsimd.tensor_copy(pool_sum[:], pool_f32[:])

            # conv22 half result staging (this core computes 64 of 128 j-rows)
            w22stage = dram.tile([64, 128], f32)
            w22full = dram.tile([128, 128], f32)

            # ============ phase 2: conv11 -> conv12 -> guided -> BN1 ========
            # Chunk-PAIR structured: per 512-chunk, taps 0-6 go scalar-IDENT
            # (PSUM downcast + bias) -> vector mul, taps 7-8 are fused vector
            # scalar_tensor_tensor reads straight from PSUM. Products land in
            # [128, 1024] pair tiles so the 8-add chain and BN1 run once per
            # pair at half the per-op overhead. ps45 is reserved up front so
            # phases 4+5 never WAR-wait on phase-2 PSUM banks.
            w22of = work.tile([128, 128], f32, tag="w22of")
            w22oT = work.tile([128, 128], f16, tag="w22oT")
            pvt = {}
            with tc.tile_pool(name="ps45", bufs=2, space="PSUM") as ps45:
              with (
                tc.tile_pool(name="ps11", bufs=2, space="PSUM") as ps11,
                tc.tile_pool(name="ps12", bufs=3, space="PSUM") as ps12,
                tc.tile_pool(name="ps22", bufs=1, space="PSUM") as ps22,
                tc.tile_pool(name="prodp1", bufs=1) as prodp1,
              ):
                def half_chunk(cidx, hoff):
                    """One 512-chunk of conv11+conv12+guided products into
                    the pv pair tiles at column offset hoff."""
                    s0, ln = chunks[cidx]
                    p11 = conv14(ps11, "p11", s0, ln, wsb["w11i"],
                                 wsb["w11gp"], wsb["w11gw"], wsb["w11gs"])
                    w11oc = convp.tile([128, NCHUNK], f16, tag="w11oc")
                    nc.scalar.activation(w11oc[:, :ln], p11[:, :ln],
                                         AF.Relu, bias=wsb["b11"][:, 0:1])

                    def xop(t):
                        di, dj = taps[t]
                        xoff = PAD + s0 + di * WP + dj - 1
                        if xoff % 2 == 0:
                            return xs[:, xoff:xoff + ln]
                        return xs2[:, xoff - 1:xoff - 1 + ln]

                    p12s = {}
                    w12ocs = {}
                    for t in range(9):
                        p12 = ps12.tile([128, NCHUNK], f32, tag="p12")
                        nc.tensor.matmul(p12[:, :ln], wsb["w12T"][:, t, :],
                                         w11oc[:, :ln], start=True, stop=True)
                        p12s[t] = p12
                        if t <= 6:  # D (downcast + bias) on scalar
                            w12oc = w12p.tile([128, NCHUNK], f16, tag="w12oc")
                            nc.scalar.activation(
                                w12oc[:, :ln], p12[:, :ln], AF.Identity,
                                bias=wsb["b12T"][:, t:t + 1])
                            w12ocs[t] = w12oc
                    for t in range(7):
                        nc.vector.tensor_mul(pvt[t][:, hoff:hoff + ln],
                                             w12ocs[t][:, :ln], xop(t))
                    for t in (7, 8):
                        nc.vector.scalar_tensor_tensor(
                            pvt[t][:, hoff:hoff + ln], p12s[t][:, :ln],
                            wsb["b12T"][:, t:t + 1], xop(t),
                            op0=mybir.AluOpType.add, op1=mybir.AluOpType.mult)

                def pair_reduce(s0, lnP):
                    """Pair-wide 8-add chain + BN1."""
                    accv = prodp.tile([128, 2 * NCHUNK], f16, tag="accv")
                    nc.vector.tensor_add(accv[:, :lnP], pvt[0][:, :lnP],
                                         pvt[1][:, :lnP])
                    for t in range(2, 9):
                        nc.vector.tensor_add(accv[:, :lnP], accv[:, :lnP],
                                             pvt[t][:, :lnP])
                    nc.scalar.activation(localp[:, s0:s0 + lnP],
                                         accv[:, :lnP], AF.Relu,
                                         bias=wsb["bias1"][:, 0:1])

                def conv22_steps(pidx):
                    for k in range(4):
                        idx = (pidx - 3) * 4 + k
                        wc = wcp.tile([128, NCHUNK], f16, tag="w22c")
                        nc.sync.dma_start(
                            wc[:],
                            w22hd.ap()[:, NCHUNK * idx:NCHUNK * (idx + 1)])
                        p22 = ps22.tile([1, NCHUNK], f32, tag="p22")
                        nc.tensor.matmul(p22[:], pool_sum[:, 0:1], wc[:],
                                         start=True, stop=True)
                        p22s = wcp.tile([1, NCHUNK], f32, tag="p22s")
                        if k % 2 == 0:
                            nc.scalar.copy(p22s[:], p22[:])
                        else:
                            nc.vector.tensor_copy(p22s[:], p22[:])
                        nc.sync.dma_start(
                            w22stage[4 * idx:4 * idx + 4, :],
                            p22s[0:1, :].rearrange("o (a b) -> o a b", a=4))

                for pidx in range(len(chunks) // 2):
                    cA, cB = 2 * pidx, 2 * pidx + 1
                    for t in range(9):
                        pvt[t] = prodp1.tile([128, 2 * NCHUNK], f16,
                                             tag=f"pv{t}", name=f"pv{t}")
                    half_chunk(cA, 0)
                    half_chunk(cB, NCHUNK)
                    pair_reduce(chunks[cA][0],
                                chunks[cA][1] + chunks[cB][1])
                    # interleaved conv22 half (16 matvec passes, pairs 3-6)
                    if 3 <= pidx <= 6:
                        conv22_steps(pidx)
                    if pidx == 6:
                        # exchange halves: [64,128]+[64,128] -> [128,128];
                        # finalize on gpsimd (keeps vector/scalar clear)
                        nc.gpsimd.collective_compute(
                            "AllGather", mybir.AluOpType.bypass,
                            replica_groups=[[0, 1], [2, 3], [4, 5], [6, 7]],
                            ins=[w22stage.opt()], outs=[w22full.opt()],
                        )
                        nc.gpsimd.dma_start(w22of[:], w22full[:])
                        nc.gpsimd.tensor_add(w22of[:], w22of[:],
                                             wsb["b22T"][:, :])
                        nc.gpsimd.tensor_copy(w22oT[:], w22of[:])

                # final solo chunk (the 388 tail)
                cidx = len(chunks) - 1
                for t in range(9):
                    pvt[t] = prodp1.tile([128, 2 * NCHUNK], f16, tag=f"pv{t}", name=f"pv{t}")
                half_chunk(cidx, 0)
                pair_reduce(chunks[cidx][0], chunks[cidx][1])

              # ===== phases 4+5: bmm -> BN2 -> conv3 -> BN3, 1-chunk lag ====
              # (phase-2 PSUM pools are closed; ps45 banks were never theirs)
              nc.gpsimd.memset(bn2o[:, 0:PADB], 0.0)
              nc.gpsimd.memset(bn2o[:, PADB + NF_A:], 0.0)
              rowsv = bn2o[:, PADB:PADB + NF_A].rearrange("p (r c) -> p r c",
                                                          c=WP)

              if True:
                def do_bmm(c):
                    s0, ln = chunks[c]
                    pb = ps45.tile([128, NCHUNK], f32, tag="p45")
                    nc.tensor.matmul(pb[:, :ln], w22oT[:, :],
                                     localp[:, s0:s0 + ln], start=True,
                                     stop=True)
                    nc.scalar.activation(bn2o[:, PADB + s0:PADB + s0 + ln],
                                         pb[:, :ln], AF.Relu,
                                         bias=wsb["bias2"][:, 0:1])
                    # zero the wrap-around pad cols inside this chunk's range
                    r0 = -(-s0 // WP)
                    r1 = -(-(s0 + ln) // WP)
                    if r1 > r0:
                        nc.gpsimd.memset(rowsv[:, r0:r1, 0:1], 0.0)
                    rr0 = -(-(s0 - (WP - 1)) // WP)
                    rr1 = -(-(s0 + ln - (WP - 1)) // WP)
                    if rr1 > rr0:
                        nc.gpsimd.memset(rowsv[:, rr0:rr1, WP - 1:WP], 0.0)
                    if c == 0:  # A-row 0: outside-image halo for top half
                        rv = bn2o[:, PADB:PADB + WP]
                        nc.gpsimd.tensor_mul(
                            rv, rv, msk[:, 0:1].to_broadcast((128, WP)))
                    if c == len(chunks) - 1:  # A-row 65
                        rv = bn2o[:, PADB + (AROWS - 1) * WP:PADB + NF_A]
                        nc.gpsimd.tensor_mul(
                            rv, rv, msk[:, 1:2].to_broadcast((128, WP)))

                def do_conv3(c):
                    s0, ln = chunks[c]
                    p3 = ps45.tile([128, NCHUNK], f32, tag="p45")
                    for t, (di, dj) in enumerate(taps):
                        off = PADB + s0 + (di - 1) * WP + (dj - 1)
                        nc.tensor.matmul(p3[:, :ln], wsb["w3T"][:, t, :],
                                         bn2o[:, off:off + ln],
                                         start=(t == 0), stop=(t == 8))
                    nc.scalar.activation(outsl[:, s0:s0 + ln], p3[:, :ln],
                                         AF.Relu, bias=wsb["bias3"][:, 0:1])

                # store owned rows [a, b) of the half-image as they complete
                ovr = outsl[:].rearrange("p (r c) -> p r c", c=WP)

                def store(a, b):
                    nc.sync.dma_start(outd.ap()[:, a:b, :],
                                      ovr[:, 1 + a:1 + b, 1:1 + W])

                stores = {5: (0, 21), 9: (21, 37), 13: (37, 53)}
                do_bmm(0)
                for c in range(1, len(chunks)):
                    do_bmm(c)
                    do_conv3(c - 1)
                    if (c - 1) in stores:
                        store(*stores[c - 1])
                do_conv3(len(chunks) - 1)
                store(53, HALF)

    nc.compile()
    return nc


def _get_nc():
    if "nc" not in _CACHE:
        _CACHE["nc"] = _build_nc()
    return _CACHE["nc"]


def kernel(**inputs) -> np.ndarray:
    from concourse.bass_utils import run_bass_kernel_spmd

    nc = _get_nc()
    params, w22halves = _prep_params(
        **{k: np.asarray(v) for k, v in inputs.items()
           if k not in ("input", "weight")})
    slabs = _prep_slabs(np.asarray(inputs["input"], np.float32),
                        np.asarray(inputs["weight"], np.float32))
    in_maps = []
    for core in range(N_CORES):
        xflat, x2, gpa, gww, mask = slabs[core]
        m = {"x": xflat, "x2": x2, "gpa": gpa, "gww": gww, "mask": mask,
             "w22half": w22halves[core % 2]}
        m.update(params)
        in_maps.append(m)
    res = run_bass_kernel_spmd(nc, in_maps, core_ids=list(range(N_CORES)),
                               **_CACHE.get("run_kwargs", {}))
    _CACHE["last_results"] = res
    out = np.empty((B, Ci, H, W), np.float32)
    for core in range(N_CORES):
        b, half = core // 2, core % 2
        out[b, :, half * HALF:(half + 1) * HALF, :] = \
            res.results[core]["out"].astype(np.float32)
    return out
